# revision 6
# baseline (speedup 1.0000x reference)
"""Optimized Trainium2 Bass kernel: fused QKV + paged attention + out-proj.

Sharding: 8 cores = 4 sequences x 2 head-groups (16 heads each), identical
host-side layout prep to the baseline. Device-side changes vs baseline:

1. Separate PSUM pools per role (qkv-accum 2 banks / scores 3 / pv 2 /
   aux 1 = 8 banks) instead of one shared 4-slot pool, so the list
   scheduler can overlap head h's exp-paced attention with head h+1's
   QKV matmuls without fighting for PSUM slots.
2. xT DMA split into 4 chunks so the first QKV matmuls start ~3us
   earlier (no 12us DMA prologue stall).
3. Engine pinning balanced: qT/kT/vT psum evacuations on ACT, softmax
   accumulation/reciprocal/normalize + transpose evacuations on DVE.
   exp stays on ACT (only engine with it).

Attention math per head (all-transposed layout, T=2560):
  scoresT[tt] (128, S) = kT_tile.T @ qT                (PE)
  probsT = exp(QK_SCALE * scoresT) in bf16             (ACT)
  accum += probsT (f32)                                (DVE; denominators)
  out_unT (D, S) += v_tile.T @ probsT                  (PE, PSUM accum)
  sums (1,S) = ones_col.T @ accum                      (PE)
  recip = 1/sums (DVE); bcast (128,S) = ones_row.T @ recip  (PE K=1)
  attnT[h] = out_unT * bcast  -> bf16                  (DVE)
"""
import numpy as np
import ml_dtypes
from contextlib import ExitStack

import concourse.bass as bass
import concourse.mybir as mybir
import concourse.tile as tile
from concourse.masks import make_identity
from concourse.bass_utils import run_bass_kernel_spmd

F32 = mybir.dt.float32
BF16 = mybir.dt.bfloat16
BF = ml_dtypes.bfloat16
Exp = mybir.ActivationFunctionType.Exp

B, S, H, D = 4, 512, 32, 128
PAGES_PER_SEQ, PAGE_SIZE = 128, 16
KV_LEN = PAGES_PER_SEQ * PAGE_SIZE          # 2048
HIDDEN = H * D                              # 4096
QK_SCALE = float(1.0 / np.sqrt(D))
HPC = 16                                    # heads per core
KT = HIDDEN // 128                          # 32 contraction tiles
THIST = KV_LEN // 128                       # 16 history t-tiles
N_CORES = 8
XCHUNK = 8                                  # kt tiles per xT DMA chunk


def _split_multi_waits(nc):
    """This walrus build rejects instructions carrying >1 sync-waits
    ("Too many sync wait commands"). Hoist extra waits onto standalone NOPs
    on the same engine immediately before the instruction."""
    for f in nc.m.functions:
        for bb in f.blocks:
            insts = bb.instructions
            i = 0
            while i < len(insts):
                ins = insts[i]
                si = ins.sync_info
                if si is not None and si.on_wait is not None and len(si.on_wait) > 1:
                    waits = list(si.on_wait)
                    new_nops = []
                    for w in waits[:-1]:
                        bi = nc.engines[ins.engine].nop(nofuse=True, hint="split_wait")
                        nop_ins = bi.ins
                        cur_list = nc.cur_bb.bb.instructions
                        assert cur_list[-1].name == nop_ins.name
                        cur_list.pop()
                        nop_ins.sync_info = mybir.SyncInfo(on_update=[], on_wait=[w])
                        new_nops.append(nop_ins)
                    si.on_wait = waits[-1:]
                    ins.sync_info = si
                    for nop_ins in reversed(new_nops):
                        insts.insert(i, nop_ins)
                        i += 1
                i += 1


def _build_attn_nc(use_mask=False, repeat=1):
    SI = S // 128
    TT = THIST + SI
    REPS = HIDDEN // 512
    T = TT * 128

    nc = bass.Bass()
    xT = nc.dram_tensor("xT", (128, KT * S), BF16, kind="ExternalInput")
    wq = nc.dram_tensor("wq", (HPC, 128, KT * 128), BF16, kind="ExternalInput")
    wk = nc.dram_tensor("wk", (HPC, 128, KT * 128), BF16, kind="ExternalInput")
    wv = nc.dram_tensor("wv", (HPC, 128, KT * 128), BF16, kind="ExternalInput")
    kh = nc.dram_tensor("kh", (HPC, 128, THIST * 128), BF16, kind="ExternalInput")
    vh = nc.dram_tensor("vh", (HPC, 128, THIST * 128), BF16, kind="ExternalInput")
    wo = nc.dram_tensor("wo", (REPS, 128, HPC * 512), BF16, kind="ExternalInput")
    if use_mask:
        maskT = nc.dram_tensor("maskT", (128, TT * S), BF16, kind="ExternalInput")
    out = nc.dram_tensor("out", (S, HIDDEN), F32, kind="ExternalOutput")

    with ExitStack() as ctx:
        tc = ctx.enter_context(tile.TileContext(nc))
        const = ctx.enter_context(tc.tile_pool(name="const", bufs=1))
        big = ctx.enter_context(tc.tile_pool(name="big", bufs=1))
        wpool = ctx.enter_context(tc.tile_pool(name="wpool", bufs=2))
        kvpool = ctx.enter_context(tc.tile_pool(name="kvpool", bufs=2))
        spool = ctx.enter_context(tc.tile_pool(name="spool", bufs=2))
        prpool = ctx.enter_context(tc.tile_pool(name="prpool", bufs=4))
        bcpool = ctx.enter_context(tc.tile_pool(name="bcpool", bufs=2))
        atpool = ctx.enter_context(tc.tile_pool(name="atpool", bufs=HPC + 2))
        wopool = ctx.enter_context(tc.tile_pool(name="wopool", bufs=2))
        outpool = ctx.enter_context(tc.tile_pool(name="outpool", bufs=4))
        psQ = ctx.enter_context(tc.tile_pool(name="psQ", bufs=2, space="PSUM"))
        psS = ctx.enter_context(tc.tile_pool(name="psS", bufs=2, space="PSUM"))
        psDen = ctx.enter_context(tc.tile_pool(name="psDen", bufs=1, space="PSUM"))
        psPV = ctx.enter_context(tc.tile_pool(name="psPV", bufs=2, space="PSUM"))
        psX = ctx.enter_context(tc.tile_pool(name="psX", bufs=1, space="PSUM"))

        ident = const.tile([128, 128], BF16, tag="ident")
        make_identity(nc, ident)
        ones_col = const.tile([128, 1], BF16, tag="ones_col")
        nc.vector.memset(ones_col, 1.0)
        ones_row = const.tile([1, 128], F32, tag="ones_row")
        nc.vector.memset(ones_row, 1.0)

        for r in range(repeat):
            wq0_sb = wpool.tile([128, KT * 128], BF16, tag="wq")
            nc.sync.dma_start(wq0_sb, wq[0])
            xT_sb = big.tile([128, KT * S], BF16, tag="xT")
            for xc in range(KT // XCHUNK):
                nc.sync.dma_start(
                    xT_sb[:, xc * XCHUNK * S:(xc + 1) * XCHUNK * S],
                    xT[:, xc * XCHUNK * S:(xc + 1) * XCHUNK * S])
            if use_mask:
                maskT_sb = big.tile([128, TT * S], BF16, tag="maskT")
                nc.sync.dma_start(maskT_sb, maskT[:, :])

            attnT = []
            for h in range(HPC):
                if h == 0:
                    wq_sb = wq0_sb
                else:
                    wq_sb = wpool.tile([128, KT * 128], BF16, tag="wq")
                    nc.sync.dma_start(wq_sb, wq[h])
                wk_sb = wpool.tile([128, KT * 128], BF16, tag="wk")
                nc.sync.dma_start(wk_sb, wk[h])
                wv_sb = wpool.tile([128, KT * 128], BF16, tag="wv")
                nc.sync.dma_start(wv_sb, wv[h])
                kT_sb = kvpool.tile([128, T], BF16, tag="kT")
                nc.sync.dma_start(kT_sb[:, :THIST * 128], kh[h])
                v_sb = kvpool.tile([128, T], BF16, tag="v")
                nc.sync.dma_start(v_sb[:, :THIST * 128], vh[h])

                ps_q = psQ.tile([128, S], F32, tag="qkv", name=f"ps_q_{r}_{h}")
                for kt in range(KT):
                    nc.tensor.matmul(
                        ps_q, lhsT=wq_sb[:, kt * 128:(kt + 1) * 128],
                        rhs=xT_sb[:, kt * S:(kt + 1) * S],
                        start=(kt == 0), stop=(kt == KT - 1))
                qT_sb = spool.tile([128, S], BF16, tag="qT")
                nc.scalar.copy(qT_sb, ps_q)

                ps_k = psQ.tile([128, S], F32, tag="qkv", name=f"ps_k_{r}_{h}")
                for kt in range(KT):
                    nc.tensor.matmul(
                        ps_k, lhsT=wk_sb[:, kt * 128:(kt + 1) * 128],
                        rhs=xT_sb[:, kt * S:(kt + 1) * S],
                        start=(kt == 0), stop=(kt == KT - 1))
                nc.scalar.copy(kT_sb[:, THIST * 128:], ps_k)

                ps_v = psQ.tile([128, S], F32, tag="qkv", name=f"ps_v_{r}_{h}")
                for kt in range(KT):
                    nc.tensor.matmul(
                        ps_v, lhsT=wv_sb[:, kt * 128:(kt + 1) * 128],
                        rhs=xT_sb[:, kt * S:(kt + 1) * S],
                        start=(kt == 0), stop=(kt == KT - 1))
                vT_sb = spool.tile([128, S], BF16, tag="vT")
                nc.scalar.copy(vT_sb, ps_v)
                for si in range(SI):
                    ps_t = psX.tile([128, 128], BF16, tag="aux",
                                    name=f"ps_t_{r}_{h}_{si}")
                    nc.tensor.transpose(ps_t, vT_sb[:, si * 128:(si + 1) * 128],
                                        ident)
                    nc.vector.tensor_copy(
                        v_sb[:, (THIST + si) * 128:(THIST + si + 1) * 128], ps_t)

                ps_pv = psPV.tile([128, S], F32, tag="pv", name=f"ps_pv_{r}_{h}")
                ps_den = psDen.tile([1, S], F32, tag="den", name=f"ps_den_{r}_{h}")
                for tt in range(TT):
                    ps_s = psS.tile([128, S], F32, tag="sc",
                                    name=f"ps_s_{r}_{h}_{tt}")
                    nc.tensor.matmul(ps_s, lhsT=kT_sb[:, tt * 128:(tt + 1) * 128],
                                     rhs=qT_sb, start=True, stop=True)
                    probsT = prpool.tile([128, S], BF16, tag="probsT")
                    if use_mask:
                        sc = prpool.tile([128, S], F32, tag="scmask")
                        nc.vector.scalar_tensor_tensor(
                            sc, ps_s, QK_SCALE, maskT_sb[:, tt * S:(tt + 1) * S],
                            op0=mybir.AluOpType.mult, op1=mybir.AluOpType.add)
                        nc.scalar.activation(probsT, sc, Exp)
                    else:
                        nc.scalar.activation(probsT, ps_s, Exp, scale=QK_SCALE)
                    nc.tensor.matmul(ps_pv, lhsT=v_sb[:, tt * 128:(tt + 1) * 128],
                                     rhs=probsT, start=(tt == 0),
                                     stop=(tt == TT - 1))
                    nc.tensor.matmul(ps_den, lhsT=ones_col, rhs=probsT,
                                     start=(tt == 0), stop=(tt == TT - 1))
                recip = spool.tile([1, S], F32, tag="recip")
                nc.vector.reciprocal(recip, ps_den)
                ps_b = psX.tile([128, S], F32, tag="aux", name=f"ps_b_{r}_{h}")
                nc.tensor.matmul(ps_b, lhsT=ones_row, rhs=recip, start=True,
                                 stop=True)
                bcast_sb = bcpool.tile([128, S], F32, tag="bcast")
                nc.vector.tensor_copy(bcast_sb, ps_b)
                at = atpool.tile([128, S], BF16, tag="attnT", name=f"at_{r}_{h}")
                nc.vector.tensor_mul(at, ps_pv, bcast_sb)
                attnT.append(at)

            PS_OUT = [psS, psS, psQ, psPV]
            for rep in range(REPS):
                wo_sb = wopool.tile([128, HPC * 512], BF16, tag="wo")
                nc.sync.dma_start(wo_sb, wo[rep])
                pss = [PS_OUT[si].tile([128, 512], F32,
                                       tag=("sc" if si < 2 else
                                            "qkv" if si == 2 else "pv"),
                                       name=f"ps_o_{r}_{rep}_{si}")
                       for si in range(SI)]
                for h in range(HPC):
                    for si in range(SI):
                        nc.tensor.matmul(
                            pss[si], lhsT=attnT[h][:, si * 128:(si + 1) * 128],
                            rhs=wo_sb[:, h * 512:(h + 1) * 512],
                            start=(h == 0), stop=(h == HPC - 1))
                for si in range(SI):
                    o_sb = outpool.tile([128, 512], F32, tag="o")
                    nc.scalar.copy(o_sb, pss[si])
                    nc.sync.dma_start(
                        out[si * 128:(si + 1) * 128, rep * 512:(rep + 1) * 512],
                        o_sb)

    _split_multi_waits(nc)
    return nc


def _make_in_maps(x, k_cache, v_cache, block_table, mask, Wqkv, Wo, use_mask):
    x = np.asarray(x, dtype=np.float32).reshape(B, S, HIDDEN)
    k_cache = np.asarray(k_cache, dtype=np.float32)
    v_cache = np.asarray(v_cache, dtype=np.float32)
    block_table = np.asarray(block_table)
    Wqkv = np.asarray(Wqkv, dtype=np.float32)
    Wo = np.asarray(Wo, dtype=np.float32)
    REPS = HIDDEN // 512

    def w_layout(w):
        # (HIDDEN, HPC*128) -> (HPC, 128, KT*128), [h,p,kt*128+m] = w[kt*128+p, h*128+m]
        return np.ascontiguousarray(
            w.reshape(KT, 128, HPC, 128).transpose(2, 1, 0, 3)
            .reshape(HPC, 128, KT * 128)).astype(BF)

    maskT_host = None
    if use_mask:
        mask = np.asarray(mask, dtype=np.float32)
        T = mask.shape[1]
        maskT_host = np.ascontiguousarray(
            mask.T.reshape(T // 128, 128, S).transpose(1, 0, 2)
            .reshape(128, (T // 128) * S)).astype(BF)

    def core_inputs(c):
        b, g = divmod(c, 2)
        hs = g * HPC * D
        pages = block_table[b]
        k_seq = np.ascontiguousarray(
            k_cache[pages].reshape(KV_LEN, H, D)[:, g * HPC:(g + 1) * HPC, :])
        v_seq = np.ascontiguousarray(
            v_cache[pages].reshape(KV_LEN, H, D)[:, g * HPC:(g + 1) * HPC, :])
        xT_host = np.ascontiguousarray(
            x[b].T.reshape(KT, 128, S).transpose(1, 0, 2)
            .reshape(128, KT * S)).astype(BF)
        kh_host = np.ascontiguousarray(
            k_seq.transpose(1, 2, 0).reshape(HPC, 128, THIST * 128)).astype(BF)
        vh_host = np.ascontiguousarray(
            v_seq.reshape(THIST, 128, HPC, 128).transpose(2, 1, 0, 3)
            .reshape(HPC, 128, THIST * 128)).astype(BF)
        wo_host = np.ascontiguousarray(
            Wo[g * HPC * D:(g + 1) * HPC * D, :]
            .reshape(HPC, 128, REPS, 512).transpose(2, 1, 0, 3)
            .reshape(REPS, 128, HPC * 512)).astype(BF)
        im = {
            "xT": xT_host,
            "wq": w_layout(Wqkv[:, hs:hs + HPC * D]),
            "wk": w_layout(Wqkv[:, HIDDEN + hs:HIDDEN + hs + HPC * D]),
            "wv": w_layout(Wqkv[:, 2 * HIDDEN + hs:2 * HIDDEN + hs + HPC * D]),
            "kh": kh_host,
            "vh": vh_host,
            "wo": wo_host,
        }
        if use_mask:
            im["maskT"] = maskT_host
        return im

    from concurrent.futures import ThreadPoolExecutor
    with ThreadPoolExecutor(max_workers=N_CORES) as ex:
        in_maps = list(ex.map(core_inputs, range(N_CORES)))
    return in_maps


_nc_cache = {}


def kernel(x, k_cache, v_cache, block_table, seq_lengths_host, kv_lengths_host,
           mask, Wqkv, Wo):
    use_mask = bool(np.any(np.asarray(mask)))
    if use_mask not in _nc_cache:
        _nc_cache[use_mask] = _build_attn_nc(use_mask=use_mask)
    nc = _nc_cache[use_mask]
    in_maps = _make_in_maps(x, k_cache, v_cache, block_table, mask, Wqkv, Wo,
                            use_mask)
    res = run_bass_kernel_spmd(nc, in_maps, core_ids=list(range(N_CORES)))
    out = np.empty((B * S, HIDDEN), np.float32)
    for b in range(B):
        out[b * S:(b + 1) * S] = res.results[2 * b]["out"] + \
            res.results[2 * b + 1]["out"]
    return out
